# revision 25
# baseline (speedup 1.0000x reference)
"""Trainium2 Bass kernel: MGNet post-processing, 8-core SPMD (H split).

Per core (rows h0..h0+128 of the 1024x2048 image):
  1. global top-32 heatmap centers: per-partition max8 -> per-core merge ->
     AllGather(32 val/idx pairs) -> redundant global merge on every core
  2. exact per-pixel argmin over 32 centers of (ys-cy)^2+(xs-cx)^2
     (ACT Square with per-k bias + DVE add/is_lt/copy_predicated)
  3. camera projection cam = (invK @ [x,y,1]) * depth
  4. surface normals: vx via shifted-AP subs, vy via PE shift-matrix matmuls,
     cross products, heights |cam.n| / (|n| + eps)
  5. exact global masked lower-median of ground-pixel heights:
     2 local histogram rounds -> per-row 24-smallest compaction (max8) ->
     AllGather pool -> f32 value bisection + exact min-above-lo extraction
  6. outputs: grouped sem (int32), filtered scaled depth, scaled cam + sem
     interleaved (H,W,4)

SBUF plane slots are shared via pool tags between phases with disjoint
lifetimes (noted inline) to stay under the per-partition budget.
"""

import numpy as np
from contextlib import ExitStack

import concourse.bass as bass
import concourse.bacc as bacc
import concourse.mybir as mybir
import concourse.tile as tile
import concourse.bass_isa as bass_isa
from concourse.bass_utils import run_bass_kernel_spmd

F32 = mybir.dt.float32
I32 = mybir.dt.int32
U32 = mybir.dt.uint32
U8 = mybir.dt.uint8
AL = mybir.AluOpType
AFT = mybir.ActivationFunctionType
AX = mybir.AxisListType
ROP = bass_isa.ReduceOp

H, W = 1024, 2048
NCORES = 8
R = H // NCORES            # 128 rows per core == partition count
K = 32                     # number of centers
CH = 512                   # chunk width (one psum bank)
NCH = W // CH
HW2 = W // 2               # k-loop / output column half
BIG = 1e30
POOLC = 32                 # median candidate slots per row
NTHR = 7                   # histogram thresholds per round
GP = NCORES * POOLC        # gathered pool columns


def _build():
    nc = bacc.Bacc("TRN2", target_bir_lowering=False, debug=False,
                   num_devices=NCORES)

    d = {}
    d["sem"] = nc.dram_tensor("sem", [R, W], I32, kind="ExternalInput")
    d["heat"] = nc.dram_tensor("heat", [R, W], F32, kind="ExternalInput")
    d["offy"] = nc.dram_tensor("offy", [R, W], F32, kind="ExternalInput")
    d["offx"] = nc.dram_tensor("offx", [R, W], F32, kind="ExternalInput")
    d["dep"] = nc.dram_tensor("dep", [R, W], F32, kind="ExternalInput")
    d["dhalo"] = nc.dram_tensor("dhalo", [2, W], F32, kind="ExternalInput")
    d["invk"] = nc.dram_tensor("invk", [1, 9], F32, kind="ExternalInput")
    d["rinfo"] = nc.dram_tensor("rinfo", [1, 4], F32, kind="ExternalInput")
    d["sem_out"] = nc.dram_tensor("sem_out", [R, W], I32, kind="ExternalOutput")
    d["dep_out"] = nc.dram_tensor("dep_out", [R, W], F32, kind="ExternalOutput")
    d["cam_out"] = nc.dram_tensor("cam_out", [R, W * 4], F32,
                                  kind="ExternalOutput")

    shift_np = np.zeros((128, 128), np.float32)
    for p in range(128):
        if p + 1 <= 127:
            shift_np[p + 1, p] = 1.0       # out[p] += cam[p+1]
        if p - 1 >= 0:
            shift_np[p - 1, p] = -1.0      # out[p] -= cam[p-1]
    halo_np = np.zeros((2, 128), np.float32)
    halo_np[0, 0] = -1.0                   # row 0: -cam_halo_top
    halo_np[1, 127] = 1.0                  # row 127: +cam_halo_bot
    id_np = np.eye(128, dtype=np.float32)
    d["shiftW"] = nc.inline_tensor(shift_np, "shiftW")
    d["haloW"] = nc.inline_tensor(halo_np, "haloW")
    d["idW"] = nc.inline_tensor(id_np, "idW")
    import ml_dtypes
    d["onesW"] = nc.inline_tensor(
        np.ones((128, 128), dtype=ml_dtypes.bfloat16), "onesW")
    d["colC"] = nc.inline_tensor(
        np.tile(np.arange(W, dtype=np.float32), (128, 1)), "colC")
    d["rowC"] = nc.inline_tensor(
        np.arange(128, dtype=np.float32).reshape(128, 1), "rowC")
    d["kvC"] = nc.inline_tensor(
        np.tile(np.arange(1, K + 1, dtype=np.float32), (128, 1)), "kvC")
    d["i7C"] = nc.inline_tensor(
        np.tile(np.arange(1, NTHR + 1, dtype=np.float32), (128, 1)), "i7C")
    d["i9C"] = nc.inline_tensor(
        np.tile(np.arange(NTHR + 2, dtype=np.float32), (128, 1)), "i9C")
    d["posC"] = nc.inline_tensor(
        np.tile(np.arange(512, dtype=np.float32), (32, 1)), "posC")

    d["tk_in"] = nc.dram_tensor("tk_in", [64], F32)
    d["tk_out"] = nc.dram_tensor("tk_out", [NCORES * 64], F32,
                                 addr_space="Shared")
    d["ct_in"] = nc.dram_tensor("ct_in", [2], F32)
    d["ct_out"] = nc.dram_tensor("ct_out", [NCORES * 2], F32,
                                 addr_space="Shared")
    d["pl_in"] = nc.dram_tensor("pl_in", [R * POOLC], F32)
    d["pl_out"] = nc.dram_tensor("pl_out", [NCORES * R * POOLC], F32,
                                 addr_space="Shared")
    for nm, sz in [("sc_a", R * 8), ("sc_b", R * 8), ("sc_w", 32),
                   ("sc_f", 32), ("sc_g", 32), ("sc_cy", 32), ("sc_cx", 32),
                   ("sc_c", 256), ("sc_d", 256), ("sc_e", 256),
                   ("sc_f2", 256), ("sc_t", 1), ("sc_p1", 256), ("sc_p2", 2)]:
        d[nm] = nc.dram_tensor(nm, [sz], F32)

    with tile.TileContext(nc) as tc:
        with ExitStack() as ctx:
            _kernel(ctx, tc, d)
    nc.compile()
    return nc


def _kernel(ctx, tc, d):
    import os
    PH = int(os.environ.get("KPHASES", "99"))
    nc = tc.nc
    grp = [list(range(NCORES))]

    P = ctx.enter_context(tc.tile_pool(name="planes", bufs=1))
    S = ctx.enter_context(tc.tile_pool(name="small", bufs=1))
    WP = ctx.enter_context(tc.tile_pool(name="work", bufs=2))
    WP1 = ctx.enter_context(tc.tile_pool(name="work1", bufs=1))
    NP = ctx.enter_context(tc.tile_pool(name="nwork", bufs=1))
    DP = ctx.enter_context(tc.tile_pool(name="drams", bufs=1, space="DRAM"))
    PSV = ctx.enter_context(tc.tile_pool(name="psumv", bufs=1, space="PSUM"))
    PSD = ctx.enter_context(tc.tile_pool(name="psumd", bufs=2, space="PSUM"))

    # ---------------- loads ----------------
    semi = P.tile([R, W], I32, tag="semi")          # slot: semi -> mw
    nc.sync.dma_start(semi[:], d["sem"][:])
    heat = P.tile([R, W], F32, tag="heat")          # slot: heat -> gnot
    nc.sync.dma_start(heat[:], d["heat"][:])
    oy = P.tile([R, W], F32, tag="oy")              # slot: oy -> mscr(xn/median)
    nc.sync.dma_start(oy[:], d["offy"][:])
    ox = P.tile([R, W], F32, tag="ox")              # slot: ox -> c4 out chunks
    nc.sync.dma_start(ox[:], d["offx"][:])
    dep = P.tile([R, W], F32, tag="dep")
    nc.sync.dma_start(dep[:], d["dep"][:])

    invk = S.tile([R, 9], F32, tag="invk")
    nc.sync.dma_start(invk[:], d["invk"][:].partition_broadcast(R))
    rinfo = S.tile([R, 4], F32, tag="rinfo")
    nc.sync.dma_start(rinfo[:], d["rinfo"][:].partition_broadcast(R))
    h0ap = rinfo[:, 0:1]
    rchap = rinfo[:, 1:2]

    shiftW = S.tile([128, 128], F32, tag="shiftW")
    nc.sync.dma_start(shiftW[:], d["shiftW"][:])
    haloW = S.tile([2, 128], F32, tag="haloW")
    nc.sync.dma_start(haloW[:], d["haloW"][:])
    idW = S.tile([128, 128], F32, tag="idW")
    nc.sync.dma_start(idW[:], d["idW"][:])
    onesW = S.tile([128, 128], mybir.dt.bfloat16, tag="onesW")
    nc.sync.dma_start(onesW[:], d["onesW"][:])

    colrep = P.tile([R, W], F32, tag="colrep")      # slot: colrep -> msk
    nc.sync.dma_start(colrep[:], d["colC"][:])
    rowc = S.tile([R, 1], F32, tag="rowc")
    nc.sync.dma_start(rowc[:], d["rowC"][:])
    rowb = S.tile([R, 1], F32, tag="rowb")
    nc.vector.tensor_scalar(rowb[:], rowc[:], h0ap, None, op0=AL.add)

    PSB = ctx.enter_context(tc.tile_pool(name="psumb", bufs=1, space="PSUM"))

    def colsum_small(src, n):
        """exact replicated column-sum for integer-valued src < 256"""
        b = S.tile([R, n], mybir.dt.bfloat16, tag="csb")
        nc.vector.tensor_copy(b[:], src)
        ps = PSB.tile([R, n], F32, tag="psb")
        nc.tensor.matmul(ps[:], onesW[:], b[:], start=True, stop=True)
        return ps

    def colsum_wide(src, n, out_tag):
        """exact replicated column-sum for integer-valued src < 2^16"""
        hi = S.tile([R, n], F32, tag="cshi")
        nc.vector.tensor_scalar(hi[:], src, 2.0 ** -8, -(0.5 - 2.0 ** -12),
                                op0=AL.mult, op1=AL.add)
        nc.vector.tensor_scalar(hi[:], hi[:], 2.0 ** 23, -(2.0 ** 23),
                                op0=AL.add, op1=AL.add)
        lo = S.tile([R, n], F32, tag="cslo")
        nc.vector.scalar_tensor_tensor(lo[:], hi[:], -256.0, src,
                                       op0=AL.mult, op1=AL.add)
        b = S.tile([R, 2 * n], mybir.dt.bfloat16, tag="csb2")
        nc.vector.tensor_copy(b[:, 0:n], lo[:])
        nc.vector.tensor_copy(b[:, n:2 * n], hi[:])
        ps = PSB.tile([R, 2 * n], F32, tag="psb")
        nc.tensor.matmul(ps[:], onesW[:], b[:], start=True, stop=True)
        pss = S.tile([R, 2 * n], F32, tag="csps")
        nc.vector.tensor_copy(pss[:], ps[:])
        out = S.tile([R, n], F32, tag=out_tag)
        nc.vector.scalar_tensor_tensor(out[:], pss[:, n:2 * n], 256.0,
                                       pss[:, 0:n], op0=AL.mult, op1=AL.add)
        return out

    def preduce_max(src, n, out_tag):
        """replicated cross-partition max via a DRAM transpose round-trip"""
        nc.sync.dma_start(d["sc_p1"][0:R * n], src)
        tr = S.tile([1, R * n], F32, tag="ptr")
        ap = d["sc_p1"][0:R * n].rearrange("(p n) -> n p", n=n).unsqueeze(0)
        nc.sync.dma_start(tr[:].rearrange("o (n p) -> o n p", n=n), ap)
        red = S.tile([1, n], F32, tag="pred")
        nc.vector.tensor_reduce(red[:], tr[:].rearrange("o (n p) -> o n p",
                                                        n=n),
                                axis=AX.X, op=AL.max)
        nc.sync.dma_start(d["sc_p2"][0:n], red[:])
        out = S.tile([R, n], F32, tag=out_tag)
        nc.sync.dma_start(out[:],
                          d["sc_p2"][0:n].unsqueeze(0).partition_broadcast(R))
        return out

    # ---------------- shifted pixel coords ----------------
    ys = P.tile([R, W], F32, tag="ys")
    nc.vector.tensor_scalar(ys[:], oy[:], rowb[:, 0:1], None, op0=AL.add)
    xs = P.tile([R, W], F32, tag="xs")
    nc.vector.tensor_tensor(xs[:], ox[:], colrep[:], op=AL.add)

    if PH < 1:
        return
    # ---------------- top-32 centers (global) ----------------
    mx = S.tile([R, 8], F32, tag="mx")
    nc.vector.max(mx[:], heat[:])
    mi = S.tile([R, 8], U32, tag="mi")
    nc.vector.max_index(mi[:], mx[:], heat[:])
    mif = S.tile([R, 8], F32, tag="mif")
    nc.vector.tensor_copy(mif[:], mi[:])
    rb2048 = S.tile([R, 1], F32, tag="rb2048")
    nc.vector.tensor_scalar(rb2048[:], rowb[:], 2048.0, None, op0=AL.mult)
    flat8 = S.tile([R, 8], F32, tag="flat8")
    nc.vector.tensor_scalar(flat8[:], mif[:], rb2048[:, 0:1], None, op0=AL.add)

    nc.sync.dma_start(d["sc_a"][:], mx[:])
    nc.sync.dma_start(d["sc_b"][:], flat8[:])
    vrow = S.tile([1, R * 8], F32, tag="vrow")
    nc.sync.dma_start(vrow[:], d["sc_a"][:])
    vwork = S.tile([1, R * 8], F32, tag="vwork")
    nc.vector.tensor_copy(vwork[:], vrow[:])
    posC = S.tile([32, 512], F32, tag="posC")
    nc.sync.dma_start(posC[:], d["posC"][:])

    w32 = S.tile([1, 32], F32, tag="w32")
    p32 = S.tile([1, 32], U32, tag="p32")
    for r in range(4):
        nc.vector.max(w32[:, 8 * r:8 * r + 8], vwork[:])
        nc.vector.max_index(p32[:, 8 * r:8 * r + 8], w32[:, 8 * r:8 * r + 8],
                            vrow[:])
        if r < 3:
            nc.vector.match_replace(vwork[:], w32[:, 8 * r:8 * r + 8],
                                    vwork[:], -BIG)
    p32f = S.tile([1, 32], F32, tag="p32f")
    nc.vector.tensor_copy(p32f[:], p32[:])
    nc.sync.dma_start(d["sc_w"][:], p32f[:])
    pT = S.tile([32, 1], F32, tag="pT")
    nc.sync.dma_start(pT[:], d["sc_w"][:])
    pT2 = S.tile([32, 1], F32, tag="pT2")
    nc.vector.tensor_scalar(pT2[:], pT[:], -512.0, None, op0=AL.add)
    fl32 = S.tile([32, 1], F32, tag="fl32")
    flh = S.tile([32, 2], F32, tag="flh")
    frep = S.tile([32, 512], F32, tag="frep")
    eqc = S.tile([32, 512], F32, tag="eqc")
    junk = S.tile([32, 512], F32, tag="junkg")
    for hh, pap in ((0, pT), (1, pT2)):
        nc.sync.dma_start(
            frep[:], d["sc_b"][512 * hh:512 * hh + 512]
            .unsqueeze(0).partition_broadcast(32))
        nc.vector.tensor_scalar(eqc[:], posC[:], pap[:, 0:1], None,
                                op0=AL.is_equal)
        nc.vector.scalar_tensor_tensor(junk[:], eqc[:], 1.0, frep[:],
                                       op0=AL.mult, op1=AL.mult,
                                       accum_out=flh[:, hh:hh + 1])
    nc.vector.tensor_tensor(fl32[:], flh[:, 0:1], flh[:, 1:2], op=AL.add)

    msg = S.tile([1, 64], F32, tag="msg")
    nc.vector.tensor_copy(msg[:, 0:32], w32[:])
    nc.sync.dma_start(d["sc_f"][:], fl32[:])
    nc.sync.dma_start(msg[:, 32:64], d["sc_f"][:])
    nc.sync.dma_start(d["tk_in"][:], msg[:])
    nc.gpsimd.collective_compute("AllGather", AL.bypass, replica_groups=grp,
                                 ins=[d["tk_in"][:]], outs=[d["tk_out"][:]])
    gall = S.tile([1, 512], F32, tag="gall")
    nc.sync.dma_start(gall[:], d["tk_out"][:])
    g3 = gall[:].rearrange("p (c t) -> p c t", c=NCORES)
    gv = S.tile([1, 256], F32, tag="gv")
    nc.vector.tensor_copy(gv[:].rearrange("p (c t) -> p c t", c=NCORES),
                          g3[:, :, 0:32])
    gf = S.tile([1, 256], F32, tag="gf")
    nc.vector.tensor_copy(gf[:].rearrange("p (c t) -> p c t", c=NCORES),
                          g3[:, :, 32:64])
    gwork = S.tile([1, 256], F32, tag="gwork")
    nc.vector.tensor_copy(gwork[:], gv[:])
    nc.sync.dma_start(d["sc_f2"][:], gf[:])
    gfrep = S.tile([32, 256], F32, tag="frep")
    nc.sync.dma_start(gfrep[:],
                      d["sc_f2"][:].unsqueeze(0).partition_broadcast(32))

    wg = S.tile([1, 32], F32, tag="wg")
    pg = S.tile([1, 32], U32, tag="pg")
    for r in range(4):
        nc.vector.max(wg[:, 8 * r:8 * r + 8], gwork[:])
        nc.vector.max_index(pg[:, 8 * r:8 * r + 8], wg[:, 8 * r:8 * r + 8],
                            gv[:])
        if r < 3:
            nc.vector.match_replace(gwork[:], wg[:, 8 * r:8 * r + 8],
                                    gwork[:], -BIG)
    pgf = S.tile([1, 32], F32, tag="pgf")
    nc.vector.tensor_copy(pgf[:], pg[:])
    nc.sync.dma_start(d["sc_g"][:], pgf[:])
    pgT = S.tile([32, 1], F32, tag="pgT")
    nc.sync.dma_start(pgT[:], d["sc_g"][:])
    eqg = S.tile([32, 256], F32, tag="eqc")
    nc.vector.tensor_scalar(eqg[:], posC[:, 0:256], pgT[:, 0:1], None,
                            op0=AL.is_equal)
    gfl = S.tile([32, 1], F32, tag="gfl")
    junk2 = S.tile([32, 256], F32, tag="junkg")

    nc.vector.scalar_tensor_tensor(junk2[:], eqg[:], 1.0, gfrep[:],
                                   op0=AL.mult, op1=AL.mult, accum_out=gfl[:])
    nc.sync.dma_start(d["sc_e"][0:32], wg[:])
    wgT = S.tile([32, 1], F32, tag="wgT")
    nc.sync.dma_start(wgT[:], d["sc_e"][0:32])

    # decode centers: cy = floor(flat/2048) exactly in f32 (flat < 2^21):
    # floor(x) = ((x - (0.5 - 2^-12)) + 2^23) - 2^23 under round-to-nearest
    cyf = S.tile([32, 1], F32, tag="cyf")
    nc.vector.tensor_scalar(cyf[:], gfl[:], 1.0 / 2048.0, -(0.5 - 2.0 ** -12),
                            op0=AL.mult, op1=AL.add)
    nc.vector.tensor_scalar(cyf[:], cyf[:], 2.0 ** 23, -(2.0 ** 23),
                            op0=AL.add, op1=AL.add)
    cxf = S.tile([32, 1], F32, tag="cxf")
    nc.vector.scalar_tensor_tensor(cxf[:], cyf[:], -2048.0, gfl[:],
                                   op0=AL.mult, op1=AL.add)
    nvmask = S.tile([32, 1], U8, tag="nvmask")
    nc.vector.tensor_scalar(nvmask[:], wgT[:], 0.0, None, op0=AL.is_le)
    big9 = S.tile([32, 1], F32, tag="big9")
    nc.vector.memset(big9[:], 1e9)
    cym = S.tile([32, 1], F32, tag="cym")
    nc.vector.tensor_copy(cym[:], cyf[:])
    nc.vector.copy_predicated(cym[:], nvmask[:], big9[:])
    cxm = S.tile([32, 1], F32, tag="cxm")
    nc.vector.tensor_copy(cxm[:], cxf[:])
    nc.vector.copy_predicated(cxm[:], nvmask[:], big9[:])

    av1 = S.tile([1, 1], F32, tag="av1")
    nc.vector.tensor_scalar(av1[:], wg[:, 0:1], 0.0, None, op0=AL.is_gt)
    nc.sync.dma_start(d["sc_t"][:], av1[:])
    anyv = S.tile([R, 1], F32, tag="anyv")
    nc.sync.dma_start(anyv[:],
                      d["sc_t"][:].unsqueeze(0).partition_broadcast(R))

    nc.sync.dma_start(d["sc_cy"][:], cym[:])
    nc.sync.dma_start(d["sc_cx"][:], cxm[:])
    negcy = S.tile([R, K], F32, tag="negcy")
    nc.sync.dma_start(negcy[:],
                      d["sc_cy"][:].unsqueeze(0).partition_broadcast(R))
    nc.vector.tensor_scalar(negcy[:], negcy[:], -1.0, None, op0=AL.mult)
    negcx = S.tile([R, K], F32, tag="negcx")
    nc.sync.dma_start(negcx[:],
                      d["sc_cx"][:].unsqueeze(0).partition_broadcast(R))
    nc.vector.tensor_scalar(negcx[:], negcx[:], -1.0, None, op0=AL.mult)
    kvals = S.tile([R, K], F32, tag="kvals")
    nc.sync.dma_start(kvals[:], d["kvC"][:])

    if PH < 2:
        return
    # ---------------- projection ----------------
    cams = []
    rths = []
    hrow = S.tile([2, 1], F32, tag="hrow")
    nc.sync.dma_start(hrow[:], d["rinfo"][0:1, 2:4])
    for c in range(3):
        rt = S.tile([R, 1], F32, tag=f"rt{c}")
        nc.vector.tensor_scalar(rt[:], rowb[:], invk[:, 3 * c + 1:3 * c + 2],
                                invk[:, 3 * c + 2:3 * c + 3],
                                op0=AL.mult, op1=AL.add)
        xn = P.tile([R, W], F32, tag="oy")          # oy dead: xn scratch slot
        nc.scalar.activation(xn[:], colrep[:], AFT.Identity, bias=rt[:, 0:1],
                             scale=invk[:, 3 * c:3 * c + 1])
        cam = P.tile([R, W], F32, tag=f"cam{c}")
        nc.vector.tensor_tensor(cam[:], xn[:], dep[:], op=AL.mult)
        cams.append(cam)
        rth = S.tile([2, 1], F32, tag=f"rth{c}")
        nc.vector.tensor_scalar(rth[:], hrow[:], invk[0:2, 3 * c + 1:3 * c + 2],
                                invk[0:2, 3 * c + 2:3 * c + 3],
                                op0=AL.mult, op1=AL.add)
        rths.append(rth)

    if PH < 3:
        return
    # ---------------- argmin over 32 centers ----------------
    rmin = P.tile([R, W], F32, tag="rmin")
    ridx = P.tile([R, W], F32, tag="ridx")          # slot: ridx -> gsem
    nc.vector.memset(ridx[:], 1.0)
    for half in range(2):
        a = half * HW2
        b = a + HW2
        for k in range(K):
            sqy = WP.tile([R, HW2], F32, tag="sqy")
            nc.scalar.activation(sqy[:], ys[:, a:b], AFT.Square,
                                 bias=negcy[:, k:k + 1], scale=1.0)
            sqx = WP.tile([R, HW2], F32, tag="sqx")
            nc.scalar.activation(sqx[:], xs[:, a:b], AFT.Square,
                                 bias=negcx[:, k:k + 1], scale=1.0)
            if k == 0:
                nc.vector.tensor_tensor(rmin[:, a:b], sqy[:], sqx[:],
                                        op=AL.add)
            else:
                d2 = WP1.tile([R, HW2], F32, tag="d2")
                nc.vector.tensor_tensor(d2[:], sqy[:], sqx[:], op=AL.add)
                lt = WP1.tile([R, HW2], U8, tag="lt")
                nc.vector.tensor_tensor(lt[:], d2[:], rmin[:, a:b],
                                        op=AL.is_lt)
                nc.vector.copy_predicated(rmin[:, a:b], lt[:], d2[:])
                nc.vector.copy_predicated(
                    ridx[:, a:b], lt[:],
                    kvals[:, k:k + 1].broadcast_to([R, HW2]))

    if PH < 4:
        return
    # ---------------- normals + heights ----------------
    hgt = P.tile([R, W], F32, tag="hgt")
    for j in range(NCH):
        a, b = j * CH, (j + 1) * CH
        vys = []
        for c in range(3):
            xnh = S.tile([2, CH], F32, tag="xnhc")
            nc.scalar.activation(xnh[:], colrep[0:2, a:b], AFT.Identity,
                                 bias=rths[c][:, 0:1],
                                 scale=invk[0:2, 3 * c:3 * c + 1])
            dhc = S.tile([2, CH], F32, tag="dhc")
            nc.sync.dma_start(dhc[:], d["dhalo"][:, a:b])
            camh = S.tile([2, CH], F32, tag=f"camhc{c}")
            nc.vector.tensor_tensor(camh[:], xnh[:], dhc[:], op=AL.mult)
            vy = PSV.tile([R, CH], F32, tag=f"vy{c}")
            nc.tensor.matmul(vy[:], shiftW[:], cams[c][:, a:b],
                             start=True, stop=False)
            nc.tensor.matmul(vy[:], haloW[:], camh[:], start=False, stop=True)
            vys.append(vy)
        vxs = []
        for c in range(3):
            vx = NP.tile([R, CH], F32, tag=f"vx{c}")
            cam = cams[c]
            if j == 0:
                nc.vector.tensor_tensor(vx[:, 1:CH], cam[:, 2:CH + 1],
                                        cam[:, 0:CH - 1], op=AL.subtract)
                nc.vector.tensor_tensor(vx[:, 0:1], cam[:, 1:2], cam[:, 0:1],
                                        op=AL.subtract)
            elif j == NCH - 1:
                nc.vector.tensor_tensor(vx[:, 0:CH - 1], cam[:, a + 1:W],
                                        cam[:, a - 1:W - 2], op=AL.subtract)
                nc.vector.tensor_tensor(vx[:, CH - 1:CH], cam[:, W - 1:W],
                                        cam[:, W - 2:W - 1], op=AL.subtract)
            else:
                nc.vector.tensor_tensor(vx[:], cam[:, a + 1:b + 1],
                                        cam[:, a - 1:b - 1], op=AL.subtract)
            vxs.append(vx)
        ns = []
        for c in range(3):
            i1, i2 = (c + 1) % 3, (c + 2) % 3
            t1 = NP.tile([R, CH], F32, tag="tA")
            nc.vector.tensor_tensor(t1[:], vxs[i1][:], vys[i2][:], op=AL.mult)
            t2 = NP.tile([R, CH], F32, tag="tB")
            nc.vector.tensor_tensor(t2[:], vxs[i2][:], vys[i1][:], op=AL.mult)
            nv = NP.tile([R, CH], F32, tag=f"nv{c}")
            nc.vector.tensor_tensor(nv[:], t1[:], t2[:], op=AL.subtract)
            ns.append(nv)
        nn = PSD.tile([R, CH], F32, tag="nn")
        for c in range(3):
            qr = NP.tile([R, CH], F32, tag="tB")
            nc.vector.tensor_tensor(qr[:], ns[c][:], ns[c][:], op=AL.mult)
            nc.tensor.matmul(nn[:], idW[:], qr[:], start=(c == 0),
                             stop=(c == 2))
        sq2 = NP.tile([R, CH], F32, tag="tA")
        nc.scalar.activation(sq2[:], nn[:], AFT.Sqrt)
        nc.vector.tensor_scalar(sq2[:], sq2[:], 1e-8, None, op0=AL.add)
        rcp = NP.tile([R, CH], F32, tag="tB")
        rscr = NP.tile([R, CH], F32, tag="tC")
        nc.vector.reciprocal_approx_accurate(rcp[:], sq2[:], rscr[:])
        # one Markstein step: r += r*(1 - s*r), brings r to ~1ulp of 1/s
        nc.vector.tensor_tensor(rscr[:], sq2[:], rcp[:], op=AL.mult)
        nc.vector.tensor_scalar(rscr[:], rscr[:], -1.0, 1.0, op0=AL.mult,
                                op1=AL.add)
        nc.vector.tensor_tensor(rscr[:], rscr[:], rcp[:], op=AL.mult)
        nc.vector.tensor_tensor(rcp[:], rcp[:], rscr[:], op=AL.add)
        # replicate reference rounding: normalize n per channel, then dot
        hdot = PSD.tile([R, CH], F32, tag="hdot")
        for c in range(3):
            nh = NP.tile([R, CH], F32, tag="tA")
            nc.vector.tensor_tensor(nh[:], ns[c][:], rcp[:], op=AL.mult)
            ph = NP.tile([R, CH], F32, tag="tC")
            nc.vector.tensor_tensor(ph[:], cams[c][:, a:b], nh[:], op=AL.mult)
            nc.tensor.matmul(hdot[:], idW[:], ph[:], start=(c == 0),
                             stop=(c == 2))
        nc.scalar.activation(hgt[:, a:b], hdot[:], AFT.Abs)

    if PH < 5:
        return
    # ---------------- masked median ----------------
    semf = P.tile([R, W], F32, tag="semf")
    nc.vector.tensor_copy(semf[:], semi[:])
    gnot = P.tile([R, W], U8, tag="heat")           # heat dead: ground-not
    cnt1 = S.tile([R, 1], F32, tag="cnt1")
    nc.vector.tensor_scalar(gnot[:], semf[:], 0.0, None, op0=AL.not_equal,
                            op1=AL.add, accum_out=cnt1[:])
    cntp = S.tile([R, 1], F32, tag="cntp")
    nc.vector.tensor_scalar(cntp[:], cnt1[:], -1.0, float(W),
                            op0=AL.mult, op1=AL.add)
    bigt = S.tile([R, 1], F32, tag="bigt")
    nc.vector.memset(bigt[:], BIG)
    msk = P.tile([R, W], F32, tag="colrep")         # colrep dead: masked hgt
    nc.vector.tensor_copy(msk[:], hgt[:])
    nc.vector.copy_predicated(msk[:], gnot[:],
                              bigt[:, 0:1].broadcast_to([R, W]))

    mn1 = S.tile([R, 1], F32, tag="mn1")
    nc.vector.tensor_reduce(mn1[:], msk[:], axis=AX.X, op=AL.min)
    mx1 = S.tile([R, 1], F32, tag="mx1")
    nc.vector.tensor_reduce(mx1[:], hgt[:], axis=AX.X, op=AL.max)
    mm2 = S.tile([R, 2], F32, tag="mm2")
    nc.vector.tensor_scalar(mm2[:, 0:1], mn1[:], -1.0, None, op0=AL.mult)
    nc.vector.tensor_copy(mm2[:, 1:2], mx1[:])
    mmr = preduce_max(mm2[:], 2, "mmr")
    lo0 = S.tile([R, 1], F32, tag="lo0")
    nc.vector.tensor_scalar(lo0[:], mmr[:, 0:1], -1.0, None, op0=AL.mult)
    hi0 = mmr[:, 1:2]

    cntl = colsum_wide(cntp[:], 1, "cntl")
    rloc = S.tile([R, 1], F32, tag="rloc")
    nc.vector.tensor_scalar(rloc[:], cntl[:], 0.5, None, op0=AL.mult)

    iot7 = S.tile([R, NTHR], F32, tag="iot7")
    nc.sync.dma_start(iot7[:], d["i7C"][:])
    mscr = P.tile([R, W], U8, tag="oy")             # median scratch plane

    i9 = S.tile([R, NTHR + 2], F32, tag="i9")
    nc.sync.dma_start(i9[:], d["i9C"][:])

    def pick(ext, idx_ap, tag):
        """out[p] = ext[p, idx[p]] via position is_eq + accumulate"""
        eqs = S.tile([R, NTHR + 2], F32, tag="eqs")
        nc.vector.tensor_scalar(eqs[:], i9[:], idx_ap, None, op0=AL.is_equal)
        out = S.tile([R, 1], F32, tag=tag)
        jk = S.tile([R, NTHR + 2], F32, tag="jks")
        nc.vector.scalar_tensor_tensor(jk[:], eqs[:], 1.0, ext[:],
                                       op0=AL.mult, op1=AL.mult,
                                       accum_out=out[:])
        return out

    def hist_round(lo_ap, hi_ap, rnd, logspace):
        ext = S.tile([R, NTHR + 2], F32, tag=f"ext{rnd}")
        nc.vector.tensor_copy(ext[:, 0:1], lo_ap)
        nc.vector.tensor_copy(ext[:, NTHR + 1:NTHR + 2], hi_ap)
        if logspace:
            # geometric thresholds t_i = lo * (hi/lo)^(i/8)
            lnl = S.tile([R, 1], F32, tag="lnl")
            nc.vector.tensor_scalar(lnl[:], lo_ap, 1e-30, None, op0=AL.max)
            nc.scalar.activation(lnl[:], lnl[:], AFT.Ln)
            lnh = S.tile([R, 1], F32, tag="lnh")
            nc.vector.tensor_scalar(lnh[:], hi_ap, 2e-30, None, op0=AL.max)
            nc.scalar.activation(lnh[:], lnh[:], AFT.Ln)
            dl = S.tile([R, 1], F32, tag="dl")
            nc.vector.tensor_tensor(dl[:], lnh[:], lnl[:], op=AL.subtract)
            nc.vector.tensor_scalar(dl[:], dl[:], 1.0 / (NTHR + 1), None,
                                    op0=AL.mult)
            le = S.tile([R, NTHR], F32, tag="le7")
            nc.vector.scalar_tensor_tensor(le[:], iot7[:], dl[:, 0:1],
                                           lnl[:, 0:1].broadcast_to([R, NTHR]),
                                           op0=AL.mult, op1=AL.add)
            nc.scalar.activation(ext[:, 1:NTHR + 1], le[:], AFT.Exp)
        else:
            stp = S.tile([R, 1], F32, tag="stp")
            nc.vector.tensor_tensor(stp[:], hi_ap, lo_ap, op=AL.subtract)
            nc.vector.tensor_scalar(stp[:], stp[:], 1.0 / (NTHR + 1), None,
                                    op0=AL.mult)
            nc.vector.scalar_tensor_tensor(ext[:, 1:NTHR + 1], iot7[:],
                                           stp[:, 0:1],
                                           lo_ap.broadcast_to([R, NTHR]),
                                           op0=AL.mult, op1=AL.add)
        cnts = S.tile([R, NTHR], F32, tag=f"cnts{rnd}")
        for i in range(NTHR):
            nc.vector.tensor_scalar(mscr[:], msk[:], ext[:, i + 1:i + 2],
                                    None, op0=AL.is_le, op1=AL.add,
                                    accum_out=cnts[:, i:i + 1])
        cntg = colsum_wide(cnts[:], NTHR, f"cntg{rnd}")
        ble = S.tile([R, NTHR], F32, tag=f"ble{rnd}")
        nc.vector.tensor_scalar(ble[:], cntg[:], rloc[:, 0:1], None,
                                op0=AL.is_le)
        bidx = S.tile([R, 1], F32, tag=f"bidx{rnd}")
        nc.vector.tensor_reduce(bidx[:], ble[:], axis=AX.X, op=AL.add)
        bl = S.tile([R, 1], F32, tag=f"bl{rnd}")
        nc.vector.tensor_scalar(bl[:], bidx[:], -1.0, 0.0, op0=AL.add,
                                op1=AL.max)
        bh = S.tile([R, 1], F32, tag=f"bh{rnd}")
        nc.vector.tensor_scalar(bh[:], bidx[:], 2.0, float(NTHR + 1),
                                op0=AL.add, op1=AL.min)
        blo = pick(ext, bl[:, 0:1], f"blo{rnd}")
        bhi = pick(ext, bh[:, 0:1], f"bhi{rnd}")
        return blo[:, 0:1], bhi[:, 0:1]

    blo_c, bhi_c = lo0[:, 0:1], hi0
    for rnd in range(1, 5):
        blo_c, bhi_c = hist_round(blo_c, bhi_c, rnd, logspace=(rnd <= 3))
    blo2, bhi2 = blo_c, bhi_c

    clo1 = S.tile([R, 1], F32, tag="clo1")
    nc.vector.tensor_scalar(mscr[:], msk[:], blo2, None, op0=AL.is_lt,
                            op1=AL.add, accum_out=clo1[:])
    pk2 = S.tile([R, 2], F32, tag="pk2")
    nc.vector.tensor_copy(pk2[:, 0:1], cntp[:])
    nc.vector.tensor_copy(pk2[:, 1:2], clo1[:])
    pk2r = colsum_wide(pk2[:], 2, "pk2r")
    nc.sync.dma_start(d["ct_in"][:], pk2r[0:1, :])
    nc.gpsimd.collective_compute("AllGather", AL.bypass, replica_groups=grp,
                                 ins=[d["ct_in"][:]], outs=[d["ct_out"][:]])
    ct16 = S.tile([R, 16], F32, tag="ct16")
    nc.sync.dma_start(ct16[:],
                      d["ct_out"][:].unsqueeze(0).partition_broadcast(R))
    totg = S.tile([R, 2], F32, tag="totg")
    nc.vector.tensor_reduce(totg[:],
                            ct16[:].rearrange("p (c t) -> p t c", c=NCORES),
                            axis=AX.X, op=AL.add)
    # m = max(floor((cnt-1)/2), 0); frac of (cnt-1)/2 in {0,.5}: -0.25 offset
    # + round-to-nearest via +-2^23 is an exact floor (cnt < 2^21)
    mtf = S.tile([R, 1], F32, tag="mtf")
    nc.vector.tensor_scalar(mtf[:], totg[:, 0:1], 0.5, -0.75,
                            op0=AL.mult, op1=AL.add)
    nc.vector.tensor_scalar(mtf[:], mtf[:], 2.0 ** 23, -(2.0 ** 23),
                            op0=AL.add, op1=AL.add)
    nc.vector.tensor_scalar(mtf[:], mtf[:], 0.0, None, op0=AL.max)
    rtgt1 = S.tile([R, 1], F32, tag="rtgt1")
    nc.vector.tensor_tensor(rtgt1[:], mtf[:], totg[:, 1:2], op=AL.subtract)
    nc.vector.tensor_scalar(rtgt1[:], rtgt1[:], 1.0, None, op0=AL.add)

    # compact: values outside (blo2, bhi2] -> BIG, take 24 smallest per row
    mw = P.tile([R, W], F32, tag="semi")            # semi dead: compact work
    nc.vector.tensor_copy(mw[:], msk[:])
    nc.vector.tensor_scalar(mscr[:], msk[:], bhi2, None, op0=AL.is_gt)
    nc.vector.copy_predicated(mw[:], mscr[:],
                              bigt[:, 0:1].broadcast_to([R, W]))
    nc.vector.tensor_scalar(mscr[:], msk[:], blo2, None, op0=AL.is_le)
    nc.vector.copy_predicated(mw[:], mscr[:],
                              bigt[:, 0:1].broadcast_to([R, W]))
    nc.vector.tensor_scalar(mw[:], mw[:], -1.0, None, op0=AL.mult)
    pool = S.tile([R, POOLC], F32, tag="pool")
    for r in range(POOLC // 8):
        nc.vector.max(pool[:, 8 * r:8 * r + 8], mw[:])
        if r < POOLC // 8 - 1:
            nc.vector.match_replace(mw[:], pool[:, 8 * r:8 * r + 8], mw[:],
                                    -BIG)
    nc.vector.tensor_scalar(pool[:], pool[:], -1.0, None, op0=AL.mult)
    nc.sync.dma_start(d["pl_in"][:], pool[:])
    nc.gpsimd.collective_compute("AllGather", AL.bypass, replica_groups=grp,
                                 ins=[d["pl_in"][:]], outs=[d["pl_out"][:]])
    gpool = S.tile([R, GP], F32, tag="gpool")
    nc.sync.dma_start(
        gpool[:].rearrange("p (c t) -> p c t", c=NCORES),
        d["pl_out"][:].rearrange("(c p t) -> p c t", c=NCORES, p=R))

    # f32 value bisection: narrow (lo, hi] until count(<=lo) == r, then the
    # answer is exactly min(pool value > lo)
    blos = S.tile([R, 1], F32, tag="blos")
    nc.vector.tensor_copy(blos[:], blo2)
    bhis = S.tile([R, 1], F32, tag="bhis")
    nc.vector.tensor_copy(bhis[:], bhi2)
    gscr = S.tile([R, GP], U8, tag="gscr")
    for it in range(42):
        mid = S.tile([R, 1], F32, tag="bmid")
        nc.vector.tensor_tensor(mid[:], blos[:], bhis[:], op=AL.add)
        nc.vector.tensor_scalar(mid[:], mid[:], 0.5, None, op0=AL.mult)
        cnb = S.tile([R, 1], F32, tag="cnb")
        nc.vector.tensor_scalar(gscr[:], gpool[:], mid[:, 0:1], None,
                                op0=AL.is_le, op1=AL.add, accum_out=cnb[:])
        cng = colsum_small(cnb[:], 1)
        ok = S.tile([R, 1], U8, tag="bok")
        nc.vector.tensor_tensor(ok[:], cng[:], rtgt1[:], op=AL.is_ge)
        nok = S.tile([R, 1], U8, tag="bnok")
        nc.vector.tensor_tensor(nok[:], cng[:], rtgt1[:], op=AL.is_lt)
        nc.vector.copy_predicated(bhis[:], ok[:], mid[:])
        nc.vector.copy_predicated(blos[:], nok[:], mid[:])
    nc.vector.tensor_scalar(gscr[:], gpool[:], blos[:, 0:1], None,
                            op0=AL.is_le)
    zz = S.tile([R, GP], F32, tag="zz")
    nc.vector.tensor_copy(zz[:], gpool[:])
    nc.vector.copy_predicated(zz[:], gscr[:],
                              bigt[:, 0:1].broadcast_to([R, GP]))
    zmn = S.tile([R, 1], F32, tag="zmn")
    nc.vector.tensor_reduce(zmn[:], zz[:], axis=AX.X, op=AL.min)
    nc.vector.tensor_scalar(zmn[:], zmn[:], -1.0, None, op0=AL.mult)
    zmx = preduce_max(zmn[:], 1, "zmx")
    camh = S.tile([R, 1], F32, tag="camh")
    nc.vector.tensor_scalar(camh[:], zmx[:], -1.0, None, op0=AL.mult)

    rec = S.tile([R, 1], F32, tag="rec")
    nc.vector.reciprocal(rec[:], camh[:])
    scale = S.tile([R, 1], F32, tag="scale")
    nc.vector.tensor_tensor(scale[:], rchap, rec[:], op=AL.mult)
    negsc = S.tile([R, 1], F32, tag="negsc")
    nc.vector.tensor_scalar(negsc[:], scale[:], -1.0, None, op0=AL.mult)

    if PH < 6:
        return
    # ---------------- grouped sem + outputs ----------------
    # compute both halves' instance-term planes BEFORE overwriting the ridx
    # slot with gsem (avoids a WAR cycle through the rotating work slots)
    ups = []
    for half in range(2):
        a = half * HW2
        b = a + HW2
        thing = WP.tile([R, HW2], F32, tag="sqy")
        nc.vector.tensor_scalar(thing[:], semf[:, a:b], 11.0, None,
                                op0=AL.is_ge)
        nc.vector.tensor_scalar(thing[:], thing[:], anyv[:, 0:1], None,
                                op0=AL.mult)
        up = WP.tile([R, HW2], F32, tag="sqx")
        nc.vector.tensor_scalar(up[:], semf[:, a:b], 999.0, None, op0=AL.mult)
        nc.vector.tensor_tensor(up[:], up[:], ridx[:, a:b], op=AL.add)
        nc.vector.tensor_tensor(up[:], up[:], thing[:], op=AL.mult)
        ups.append(up)
    gsem = P.tile([R, W], F32, tag="ridx")          # overwrites ridx slot
    for half in range(2):
        a = half * HW2
        b = a + HW2
        nc.vector.tensor_tensor(gsem[:, a:b], semf[:, a:b], ups[half][:],
                                op=AL.add)
        semo = WP1.tile([R, HW2], I32, tag="d2")
        nc.vector.tensor_copy(semo[:], gsem[:, a:b])
        nc.sync.dma_start(d["sem_out"][:, a:b], semo[:])

        dsc = WP.tile([R, HW2], F32, tag="sqy")
        nc.vector.tensor_scalar(dsc[:], dep[:, a:b], negsc[:, 0:1], None,
                                op0=AL.mult)
        f10 = WP1.tile([R, HW2], F32, tag="lt")
        nc.vector.tensor_scalar(f10[:], gsem[:, a:b], 10.0, None,
                                op0=AL.is_equal)
        f19 = WP.tile([R, HW2], F32, tag="sqx")
        nc.vector.tensor_scalar(f19[:], gsem[:, a:b], 19.0, None,
                                op0=AL.is_equal)
        nc.vector.tensor_tensor(f10[:], f10[:], f19[:], op=AL.add)
        dout = WP1.tile([R, HW2], F32, tag="d2")
        nc.vector.scalar_tensor_tensor(dout[:], f10[:], 1.0, dsc[:],
                                       op0=AL.subtract, op1=AL.mult)
        nc.sync.dma_start(d["dep_out"][:, a:b], dout[:])

    for j in range(NCH):
        a, b = j * CH, (j + 1) * CH
        c4 = P.tile([R, CH, 4], F32, tag="ox")      # ox dead: interleave buf
        for c in range(3):
            nc.vector.tensor_scalar(c4[:, :, c], cams[c][:, a:b],
                                    scale[:, 0:1], None, op0=AL.mult)
        nc.vector.tensor_copy(c4[:, :, 3], gsem[:, a:b])
        nc.sync.dma_start(d["cam_out"][:, a * 4:b * 4], c4[:])


_CACHE = {}


def _program():
    if "nc" not in _CACHE:
        _CACHE["nc"] = _build()
    return _CACHE["nc"]


def make_in_maps(sem_seg, center_heatmap, offsets, depth_logits,
                 inverse_camera_matrix, real_camera_height):
    sem = np.ascontiguousarray(np.asarray(sem_seg).reshape(H, W)).astype(np.int32)
    heat = np.ascontiguousarray(
        np.asarray(center_heatmap, np.float32).reshape(H, W))
    offs = np.asarray(offsets, np.float32).reshape(2, H, W)
    dept = np.ascontiguousarray(
        np.asarray(depth_logits, np.float32).reshape(H, W))
    invk = np.asarray(inverse_camera_matrix, np.float32).reshape(1, 9)
    rch = float(np.asarray(real_camera_height, np.float32).reshape(-1)[0])
    in_maps = []
    for c in range(NCORES):
        h0 = c * R
        top, bot = max(h0 - 1, 0), min(h0 + R, H - 1)
        in_maps.append({
            "sem": sem[h0:h0 + R],
            "heat": heat[h0:h0 + R],
            "offy": np.ascontiguousarray(offs[0, h0:h0 + R]),
            "offx": np.ascontiguousarray(offs[1, h0:h0 + R]),
            "dep": dept[h0:h0 + R],
            "dhalo": np.ascontiguousarray(dept[[top, bot]]),
            "invk": invk,
            "rinfo": np.array([[h0, rch, top, bot]], np.float32),
        })
    return in_maps


def assemble(results):
    sem_o = np.concatenate([results[c]["sem_out"] for c in range(NCORES)],
                           axis=0).reshape(1, H, W).astype(np.int32)
    dep_o = np.concatenate([results[c]["dep_out"] for c in range(NCORES)],
                           axis=0).reshape(1, H, W).astype(np.float32)
    cam_o = np.concatenate([results[c]["cam_out"] for c in range(NCORES)],
                           axis=0).reshape(H, W, 4).astype(np.float32)
    return sem_o, dep_o, cam_o


def kernel(sem_seg, center_heatmap, offsets, depth_logits,
           inverse_camera_matrix, real_camera_height):
    nc = _program()
    in_maps = make_in_maps(sem_seg, center_heatmap, offsets, depth_logits,
                           inverse_camera_matrix, real_camera_height)
    res = run_bass_kernel_spmd(nc, in_maps, list(range(NCORES)))
    return assemble(res.results)
